# revision 9
# baseline (speedup 1.0000x reference)
"""Trainium2 Bass kernel for nn_CrossModalFusion (dense cross-modal transformer).

Contract: kernel(**inputs) takes FULL inputs (B=64), shards batch across 8
NeuronCores (pure data parallel, 8 samples/core), runs one SPMD Bass program,
gathers to the full (64, 258, 256) output.

Layout strategy: activations are kept feature-major ("F-major": feature dim on
SBUF partitions, tokens on the free dim) so every linear contracts over
partitions. fp32r (fp32 rounded to 11-bit mantissa) is used for all matmul
operands - full PE rate with ~1e-4 relative rounding.
"""

import sys

sys.path.insert(0, "/opt/trn_rl_repo")

import math
import numpy as np

import concourse.bass as bass  # noqa: F401  (bass must import before bacc)
import concourse.bacc as bacc
import concourse.tile as tile
from concourse import mybir
from concourse.bass_utils import run_bass_kernel_spmd

F32 = mybir.dt.float32
F32R = mybir.dt.float32r
AF = mybir.ActivationFunctionType
ALU = mybir.AluOpType
AX = mybir.AxisListType

N_CORES = 8
B = 64
BL = B // N_CORES          # 8 samples per core
NPATCH = 256
LT = 256
L = 256                    # tokens per sample (both modalities)
T = BL * L                 # 2048 tokens per core
D = 768
DC = D // 128              # 6 feature chunks
J = 256
JC = J // 128              # 2 joint chunks
H = 4
DH = D // H                # 192
EPS = 1e-5

# (chunk, partition_offset, size) pieces of each 192-row head in the 6x128 grid
PIECES = {
    0: [(0, 0, 128), (1, 0, 64)],
    1: [(1, 64, 64), (2, 0, 128)],
    2: [(3, 0, 128), (4, 0, 64)],
    3: [(4, 64, 64), (5, 0, 128)],
}
# joint heads (dh=64): head h -> (chunk, part_offset)
JPIECE = {h: (h // 2, (h % 2) * 64) for h in range(H)}

_prog_cache = {}


def _pack_wT(w):
    # w [dout, din] -> [din/128, 128, dout] (W^T chunked for lhsT use)
    dout, din = w.shape
    return np.ascontiguousarray(w.T.reshape(din // 128, 128, dout), dtype=np.float32)


def _pack_b(b):
    # b [dout] -> [128, dout/128]
    dout = b.shape[0]
    return np.ascontiguousarray(b.reshape(dout // 128, 128).T, dtype=np.float32)


def _pack_xT(x):
    # x [BL, L, D] -> [DC, 128, BL*L]
    bl, l, d = x.shape
    return np.ascontiguousarray(x.reshape(bl * l, d).T.reshape(d // 128, 128, bl * l),
                                dtype=np.float32)


def _ln_partition(nc, sp, pq, r, nch, ncols, ones, g, b, out, tag, epst=None, zerot=None):
    """LayerNorm over the partition (feature) dim of F-major r [128, nch, ncols].

    g/b are [128, nch] per-feature tiles; out [128, nch, ncols].
    Uses ones-matmul partition sums, rstd = exp(-0.5*ln(var+eps)).
    """
    nfeat = float(nch * 128)
    sq = sp.tile([128, nch, ncols], F32R, tag=tag + "_sq")
    nc.vector.tensor_tensor(sq[:], r[:], r[:], ALU.mult)
    ps_sum = pq.tile([1, ncols], F32, tag="stat")
    for k in range(nch):
        nc.tensor.matmul(ps_sum[:], ones[:, 0:1], r[:, k, :],
                         start=(k == 0), stop=(k == nch - 1))
    ps_ssq = pq.tile([1, ncols], F32, tag="stat")
    for k in range(nch):
        nc.tensor.matmul(ps_ssq[:], ones[:, 0:1], sq[:, k, :],
                         start=(k == 0), stop=(k == nch - 1))
    mean = sp.tile([1, ncols], F32, tag=tag + "_mean")
    nc.vector.tensor_scalar_mul(mean[:], ps_sum[:], 1.0 / nfeat)
    msq = sp.tile([1, ncols], F32, tag=tag + "_msq")
    nc.vector.tensor_tensor(msq[:], mean[:], mean[:], ALU.mult)
    var = sp.tile([1, ncols], F32, tag=tag + "_var")
    nc.vector.scalar_tensor_tensor(var[:], ps_ssq[:], 1.0 / nfeat, msq[:],
                                   ALU.mult, ALU.subtract)
    lnv = sp.tile([1, ncols], F32, tag=tag + "_lnv")
    nc.scalar.activation(lnv[:], var[:], AF.Ln, bias=epst[0:1, 0:1], scale=1.0)
    rstd = sp.tile([1, ncols], F32R, tag=tag + "_rstd")
    with nc.allow_low_precision(reason="ln rstd"):
        nc.scalar.activation(rstd[:], lnv[:], AF.Exp, bias=zerot[0:1, 0:1], scale=-0.5)
    b2 = sp.tile([1, ncols], F32R, tag=tag + "_b2")
    with nc.allow_low_precision(reason="ln b2"):
        nc.vector.scalar_tensor_tensor(b2[:], mean[:], -1.0, rstd[:],
                                       ALU.mult, ALU.mult)
    ps_m = pq.tile([128, ncols], F32, tag="bc")
    nc.tensor.matmul(ps_m[:], ones[0:1, :], rstd[:], start=True, stop=True)
    ps_b = pq.tile([128, ncols], F32, tag="bc")
    nc.tensor.matmul(ps_b[:], ones[0:1, :], b2[:], start=True, stop=True)
    tmp = sq  # reuse
    for m in range(nch):
        nc.vector.tensor_tensor(tmp[:, m, :], r[:, m, :], ps_m[:], ALU.mult)
        nc.vector.tensor_tensor(tmp[:, m, :], tmp[:, m, :], ps_b[:], ALU.add)
        nc.scalar.activation(out[:, m, :], tmp[:, m, :], AF.Identity,
                             bias=b[:, m:m + 1], scale=g[:, m:m + 1])


def _build_program(alpha_txt, alpha_ip, alpha_ig):
    nc = bacc.Bacc("TRN2", target_bir_lowering=False, debug=False,
                   num_devices=N_CORES)

    def din(name, shape, dt=F32R):
        return nc.dram_tensor(name, shape, dt, kind="ExternalInput")

    # activations
    xt = din("xt", [DC, 128, T])
    xi = din("xi", [DC, 128, T])
    xg = din("xg", [DC, 128, BL])
    # enhancer params
    enh_p = {}
    for pre in ("t", "i"):
        enh_p[pre] = dict(
            wq=din(f"{pre}_wq", [DC, 128, D]), wk=din(f"{pre}_wk", [DC, 128, D]),
            wv=din(f"{pre}_wv", [DC, 128, D]), wo=din(f"{pre}_wo", [DC, 128, D]),
            bq=din(f"{pre}_bq", [128, DC], F32), bo=din(f"{pre}_bo", [128, DC], F32),
            pos=din(f"{pre}_pos", [DC, 128, L]),
            lng=din(f"{pre}_lng", [128, DC], F32), lnb=din(f"{pre}_lnb", [128, DC], F32),
        )
    g_w = din("g_w", [DC, 128, D])
    g_bo = din("g_bo", [128, DC], F32)
    g_pos = din("g_pos", [DC, 128, BL])  # pre-replicated over BL on host
    g_lng = din("g_lng", [128, DC], F32)
    g_lnb = din("g_lnb", [128, DC], F32)
    # cross projections 768->256 (biases folded into joint mha biases)
    c_qt = din("c_qt", [DC, 128, J]); c_kt = din("c_kt", [DC, 128, J])
    c_vt = din("c_vt", [DC, 128, J]); c_ki = din("c_ki", [DC, 128, J])
    c_vi = din("c_vi", [DC, 128, J]); c_qi = din("c_qi", [DC, 128, J])
    c_pp = din("c_pp", [DC, 128, J]); c_pp_b = din("c_pp_b", [128, JC], F32)
    c_gp = din("c_gp", [DC, 128, J]); c_gp_b = din("c_gp_b", [128, JC], F32)
    c_tp = din("c_tp", [DC, 128, J]); c_tp_b = din("c_tp_b", [128, JC], F32)
    # joint mhas
    jnt = {}
    for pre in ("a", "b"):  # a = t2i, b = i2t
        jnt[pre] = dict(
            wq=din(f"{pre}_wq", [JC, 128, J]), wk=din(f"{pre}_wk", [JC, 128, J]),
            wv=din(f"{pre}_wv", [JC, 128, J]), wo=din(f"{pre}_wo", [JC, 128, J]),
            bq=din(f"{pre}_bq", [128, JC], F32), bo=din(f"{pre}_bo", [128, JC], F32),
        )
    ln1g = din("ln1g", [128, JC], F32); ln1b = din("ln1b", [128, JC], F32)
    ln2g = din("ln2g", [128, JC], F32); ln2b = din("ln2b", [128, JC], F32)
    ones_d = din("ones", [128, 128])
    consts_d = din("consts", [1, 2], F32)  # [eps, 0]
    identr_d = din("identr", [128, 128])

    # DRAM intermediates (cross-projection outputs, F-major [JC,128,T])
    dr = {nm: nc.dram_tensor(nm, [JC, 128, T], F32R)
          for nm in ("QT", "KT", "VT", "KI", "VI", "QI", "PP")}
    out_d = nc.dram_tensor("out", [BL, 1 + NPATCH + 1, J], F32,
                           kind="ExternalOutput")

    with tile.TileContext(nc) as tc:
        with tc.tile_pool(name="glob", bufs=1) as gp, \
             tc.tile_pool(name="gps", bufs=2, space="PSUM") as pq:
            ones = gp.tile([128, 128], F32R)
            nc.sync.dma_start(ones[:], ones_d[:])
            identr = gp.tile([128, 128], F32R)
            nc.sync.dma_start(identr[:], identr_d[:])
            consts = gp.tile([1, 2], F32)
            nc.sync.dma_start(consts[:], consts_d[:])
            epst = consts[:, 0:1]
            zerot = consts[:, 1:2]
            enh_first = gp.tile([128, DC, BL], F32R)   # txt first tokens
            tcls = gp.tile([128, JC, BL], F32R)        # txt_proj of first tokens
            x1a = gp.tile([128, JC, BL], F32R)         # img_global_proj(ig_enh)
            attm_a = gp.tile([128, JC, BL], F32R)      # t2i attO q-sums
            attm_b = gp.tile([128, JC, BL], F32R)      # i2t out q-sums
            z1 = gp.tile([128, JC, BL], F32R)
            z2 = gp.tile([128, JC, BL], F32R)
            x1t = gp.tile([128, JC, BL], F32R)
            x2t = gp.tile([128, JC, BL], F32R)

            # ---------------- stage A/B: the two 768-dim self-attn enhancers
            for pre, src, alpha, cross_list in (
                ("t", xt, alpha_txt, [(c_qt, dr["QT"]), (c_kt, dr["KT"]),
                                      (c_vt, dr["VT"])]),
                ("i", xi, alpha_ip, [(c_ki, dr["KI"]), (c_vi, dr["VI"]),
                                     (c_qi, dr["QI"]), (c_pp, dr["PP"])]),
            ):
                p = enh_p[pre]
                with tc.tile_pool(name=f"w_{pre}", bufs=1) as wp, \
                     tc.tile_pool(name=f"s_{pre}", bufs=1) as sp, \
                     tc.tile_pool(name=f"pp_{pre}", bufs=3, space="PSUM") as pp:
                    wq = wp.tile([128, DC, D], F32R)
                    nc.sync.dma_start(wq[:], p["wq"].ap().rearrange("k p m -> p k m"))
                    wk = wp.tile([128, DC, D], F32R)
                    nc.sync.dma_start(wk[:], p["wk"].ap().rearrange("k p m -> p k m"))
                    wv = wp.tile([128, DC, D], F32R)
                    nc.sync.dma_start(wv[:], p["wv"].ap().rearrange("k p m -> p k m"))
                    wo = wp.tile([128, DC, D], F32R)
                    nc.sync.dma_start(wo[:], p["wo"].ap().rearrange("k p m -> p k m"))
                    pos = wp.tile([128, DC, L], F32R)
                    nc.sync.dma_start(pos[:], p["pos"].ap().rearrange("k p t -> p k t"))
                    bq = wp.tile([128, DC], F32); nc.sync.dma_start(bq[:], p["bq"].ap())
                    bo = wp.tile([128, DC], F32); nc.sync.dma_start(bo[:], p["bo"].ap())
                    lng = wp.tile([128, DC], F32); nc.sync.dma_start(lng[:], p["lng"].ap())
                    lnb = wp.tile([128, DC], F32); nc.sync.dma_start(lnb[:], p["lnb"].ap())
                    cw = []
                    for ci, (cwd, _) in enumerate(cross_list):
                        t_ = wp.tile([128, DC, J], F32R, tag=f"cw{ci}")
                        nc.sync.dma_start(t_[:], cwd.ap().rearrange("k p m -> p k m"))
                        cw.append(t_)
                    ppb = None
                    if pre == "i":
                        ppb = wp.tile([128, JC], F32)
                        nc.sync.dma_start(ppb[:], c_pp_b.ap())

                    for s in range(BL):
                        sl = slice(s * L, (s + 1) * L)
                        x = sp.tile([128, DC, L], F32R, tag="x", bufs=2)
                        nc.sync.dma_start(
                            x[:], src[:, :, sl].rearrange("k p t -> p k t"))
                        nc.vector.tensor_tensor(x[:], x[:], pos[:], ALU.add)

                        q = sp.tile([128, DC, L], F32R, tag="q", bufs=1)
                        for m in range(DC):
                            ps_ = pp.tile([128, L], F32, tag="mm")
                            for k in range(DC):
                                nc.tensor.matmul(
                                    ps_[:], wq[:, k, m * 128:(m + 1) * 128],
                                    x[:, k, :], start=(k == 0), stop=(k == DC - 1))
                            nc.scalar.activation(q[:, m, :], ps_[:], AF.Identity,
                                                 bias=bq[:, m:m + 1])
                        kt = sp.tile([128, DC, L], F32R, tag="k", bufs=1)
                        for m in range(DC):
                            ps_ = pp.tile([128, L], F32, tag="mm")
                            for k in range(DC):
                                nc.tensor.matmul(
                                    ps_[:], wk[:, k, m * 128:(m + 1) * 128],
                                    x[:, k, :], start=(k == 0), stop=(k == DC - 1))
                            nc.scalar.copy(kt[:, m, :], ps_[:])
                        v = sp.tile([128, 2, D], F32R, tag="v", bufs=1)
                        for tkc in range(2):
                            for dsl in range(2):
                                ps_ = pp.tile([128, 384], F32, tag="mm")
                                for k in range(DC):
                                    nc.tensor.matmul(
                                        ps_[:], x[:, k, tkc * 128:(tkc + 1) * 128],
                                        wv[:, k, dsl * 384:(dsl + 1) * 384],
                                        start=(k == 0), stop=(k == DC - 1))
                                nc.scalar.copy(v[:, tkc, dsl * 384:(dsl + 1) * 384],
                                               ps_[:])

                        attO = sp.tile([128, DC, L], F32R, tag="scr6", bufs=1)
                        for h in range(H):
                            pieces = PIECES[h]
                            expS = sp.tile([128, 2, L], F32R, tag="expS", bufs=2)
                            for kc in range(2):
                                ps_s = pp.tile([128, L], F32, tag="mm")
                                for pi, (c, po, sz) in enumerate(pieces):
                                    nc.tensor.matmul(
                                        ps_s[:],
                                        kt[po:po + sz, c, kc * 128:(kc + 1) * 128],
                                        q[po:po + sz, c, :],
                                        start=(pi == 0), stop=(pi == len(pieces) - 1))
                                nc.scalar.activation(expS[:, kc, :], ps_s[:], AF.Exp)
                            ps_cs = pq.tile([1, L], F32, tag="stat")
                            for kc in range(2):
                                nc.tensor.matmul(ps_cs[:], ones[:, 0:1],
                                                 expS[:, kc, :],
                                                 start=(kc == 0), stop=(kc == 1))
                            rec = sp.tile([1, L], F32R, tag="rec", bufs=2)
                            with nc.allow_low_precision(reason="softmax recip"):
                                nc.vector.reciprocal(rec[:], ps_cs[:])
                            ps_bc = pq.tile([128, L], F32, tag="bc")
                            nc.tensor.matmul(ps_bc[:], ones[0:1, :], rec[:],
                                             start=True, stop=True)
                            for kc in range(2):
                                nc.vector.tensor_tensor(expS[:, kc, :],
                                                        expS[:, kc, :], ps_bc[:],
                                                        ALU.mult)
                            for (c, po, sz) in pieces:
                                ps_av = pp.tile([128, L], F32, tag="mm")
                                for kc in range(2):
                                    nc.tensor.matmul(
                                        ps_av[0:sz, :],
                                        v[:, kc, c * 128 + po:c * 128 + po + sz],
                                        expS[:, kc, :],
                                        start=(kc == 0), stop=(kc == 1))
                                nc.scalar.copy(attO[po:po + sz, c, :], ps_av[0:sz, :])

                        r = sp.tile([128, DC, L], F32R, tag="r", bufs=1)
                        for m in range(DC):
                            ps_ = pp.tile([128, L], F32, tag="mm")
                            for k in range(DC):
                                nc.tensor.matmul(
                                    ps_[:], wo[:, k, m * 128:(m + 1) * 128],
                                    attO[:, k, :], start=(k == 0), stop=(k == DC - 1))
                            if alpha == 1.0:
                                nc.vector.scalar_tensor_tensor(
                                    r[:, m, :], ps_[:], bo[:, m:m + 1], x[:, m, :],
                                    ALU.add, ALU.add)
                            else:
                                nc.vector.tensor_scalar_mul(x[:, m, :], x[:, m, :],
                                                            float(alpha))
                                nc.vector.scalar_tensor_tensor(
                                    r[:, m, :], ps_[:], bo[:, m:m + 1], x[:, m, :],
                                    ALU.add, ALU.add)

                        enh = sp.tile([128, DC, L], F32R, tag="enh", bufs=1)
                        _ln_partition(nc, sp, pq, r, DC, L, ones, lng, lnb, enh,
                                      tag="ln", epst=epst, zerot=zerot)

                        if pre == "t":
                            nc.vector.tensor_copy(enh_first[:, :, s:s + 1],
                                                  enh[:, :, 0:1])
                        for ci, (_, cdram) in enumerate(cross_list):
                            cg = sp.tile([128, JC, L], F32R, tag=f"cr{ci}", bufs=2)
                            for m in range(JC):
                                ps_ = pp.tile([128, L], F32, tag="mm")
                                for k in range(DC):
                                    nc.tensor.matmul(
                                        ps_[:], cw[ci][:, k, m * 128:(m + 1) * 128],
                                        enh[:, k, :], start=(k == 0),
                                        stop=(k == DC - 1))
                                if pre == "i" and ci == 3:
                                    nc.scalar.activation(cg[:, m, :], ps_[:],
                                                         AF.Identity,
                                                         bias=ppb[:, m:m + 1])
                                else:
                                    nc.scalar.copy(cg[:, m, :], ps_[:])
                            nc.sync.dma_start(
                                cdram[:, :, sl].rearrange("k p t -> p k t"), cg[:])

            # ---------------- stage C: img_global enhancer (L=1) + small projs
            with tc.tile_pool(name="w_g", bufs=1) as wp, \
                 tc.tile_pool(name="s_g", bufs=1) as sp, \
                 tc.tile_pool(name="pp_g", bufs=2, space="PSUM") as pp:
                gw = wp.tile([128, DC, D], F32R)
                nc.sync.dma_start(gw[:], g_w.ap().rearrange("k p m -> p k m"))
                gbo = wp.tile([128, DC], F32); nc.sync.dma_start(gbo[:], g_bo.ap())
                gpos = wp.tile([128, DC, BL], F32R)
                nc.sync.dma_start(gpos[:], g_pos.ap().rearrange("k p t -> p k t"))
                glng = wp.tile([128, DC], F32); nc.sync.dma_start(glng[:], g_lng.ap())
                glnb = wp.tile([128, DC], F32); nc.sync.dma_start(glnb[:], g_lnb.ap())
                gpw = wp.tile([128, DC, J], F32R)
                nc.sync.dma_start(gpw[:], c_gp.ap().rearrange("k p m -> p k m"))
                gpb = wp.tile([128, JC], F32); nc.sync.dma_start(gpb[:], c_gp_b.ap())
                tpw = wp.tile([128, DC, J], F32R)
                nc.sync.dma_start(tpw[:], c_tp.ap().rearrange("k p m -> p k m"))
                tpb = wp.tile([128, JC], F32); nc.sync.dma_start(tpb[:], c_tp_b.ap())

                xgt = sp.tile([128, DC, BL], F32R, tag="xg")
                nc.sync.dma_start(xgt[:], xg.ap().rearrange("k p t -> p k t"))
                nc.vector.tensor_tensor(xgt[:], xgt[:], gpos[:], ALU.add)
                rg = sp.tile([128, DC, BL], F32R, tag="rg")
                for m in range(DC):
                    ps_ = pp.tile([128, BL], F32, tag="mm")
                    for k in range(DC):
                        nc.tensor.matmul(ps_[:], gw[:, k, m * 128:(m + 1) * 128],
                                         xgt[:, k, :], start=(k == 0),
                                         stop=(k == DC - 1))
                    if alpha_ig == 1.0:
                        nc.vector.scalar_tensor_tensor(
                            rg[:, m, :], ps_[:], gbo[:, m:m + 1], xgt[:, m, :],
                            ALU.add, ALU.add)
                    else:
                        nc.vector.tensor_scalar_mul(xgt[:, m, :], xgt[:, m, :],
                                                    float(alpha_ig))
                        nc.vector.scalar_tensor_tensor(
                            rg[:, m, :], ps_[:], gbo[:, m:m + 1], xgt[:, m, :],
                            ALU.add, ALU.add)
                geh = sp.tile([128, DC, BL], F32R, tag="geh")
                _ln_partition(nc, sp, pq, rg, DC, BL, ones, glng, glnb, geh,
                              tag="lng", epst=epst, zerot=zerot)
                for m in range(JC):
                    ps_ = pp.tile([128, BL], F32, tag="mm")
                    for k in range(DC):
                        nc.tensor.matmul(ps_[:], gpw[:, k, m * 128:(m + 1) * 128],
                                         geh[:, k, :], start=(k == 0),
                                         stop=(k == DC - 1))
                    nc.scalar.activation(x1a[:, m, :], ps_[:], AF.Identity,
                                         bias=gpb[:, m:m + 1])
                # txt_cls projection from collected first tokens
                for m in range(JC):
                    ps_ = pp.tile([128, BL], F32, tag="mm")
                    for k in range(DC):
                        nc.tensor.matmul(ps_[:], tpw[:, k, m * 128:(m + 1) * 128],
                                         enh_first[:, k, :], start=(k == 0),
                                         stop=(k == DC - 1))
                    nc.scalar.activation(tcls[:, m, :], ps_[:], AF.Identity,
                                         bias=tpb[:, m:m + 1])

            # ---------------- stage E1: t2i cross attention (mean only)
            with tc.tile_pool(name="w_a", bufs=1) as wp, \
                 tc.tile_pool(name="s_a", bufs=1) as sp, \
                 tc.tile_pool(name="pp_a", bufs=3, space="PSUM") as pp:
                jp = jnt["a"]
                wqa = wp.tile([128, JC, J], F32R)
                nc.sync.dma_start(wqa[:], jp["wq"].ap().rearrange("k p m -> p k m"))
                wka = wp.tile([128, JC, J], F32R)
                nc.sync.dma_start(wka[:], jp["wk"].ap().rearrange("k p m -> p k m"))
                wva = wp.tile([128, JC, J], F32R)
                nc.sync.dma_start(wva[:], jp["wv"].ap().rearrange("k p m -> p k m"))
                woa = wp.tile([128, JC, J], F32R)
                nc.sync.dma_start(woa[:], jp["wo"].ap().rearrange("k p m -> p k m"))
                bqa = wp.tile([128, JC], F32); nc.sync.dma_start(bqa[:], jp["bq"].ap())
                boa = wp.tile([128, JC], F32); nc.sync.dma_start(boa[:], jp["bo"].ap())
                qin = sp.tile([128, JC, T], F32R, tag="qin")
                nc.sync.dma_start(qin[:], dr["QT"].ap().rearrange("k p t -> p k t"))
                kin = sp.tile([128, JC, T], F32R, tag="kin")
                nc.sync.dma_start(kin[:], dr["KI"].ap().rearrange("k p t -> p k t"))
                vin = sp.tile([128, JC, T], F32R, tag="vin")
                nc.sync.dma_start(vin[:], dr["VI"].ap().rearrange("k p t -> p k t"))

                qj = sp.tile([128, JC, T], F32R, tag="qj")
                kj = sp.tile([128, JC, T], F32R, tag="kj")
                for m in range(JC):
                    for t4 in range(4):
                        tsl = slice(t4 * 512, (t4 + 1) * 512)
                        ps_ = pp.tile([128, 512], F32, tag="mm")
                        for k in range(JC):
                            nc.tensor.matmul(ps_[:], wqa[:, k, m * 128:(m + 1) * 128],
                                             qin[:, k, tsl], start=(k == 0),
                                             stop=(k == JC - 1))
                        nc.scalar.activation(qj[:, m, tsl], ps_[:], AF.Identity,
                                             bias=bqa[:, m:m + 1])
                        ps_ = pp.tile([128, 512], F32, tag="mm")
                        for k in range(JC):
                            nc.tensor.matmul(ps_[:], wka[:, k, m * 128:(m + 1) * 128],
                                             kin[:, k, tsl], start=(k == 0),
                                             stop=(k == JC - 1))
                        nc.scalar.copy(kj[:, m, tsl], ps_[:])
                vj = sp.tile([128, 2 * BL, J], F32R, tag="vj")
                for tkc in range(2 * BL):
                    ps_ = pp.tile([128, J], F32, tag="mm")
                    for k in range(JC):
                        nc.tensor.matmul(ps_[:], vin[:, k, tkc * 128:(tkc + 1) * 128],
                                         wva[:, k, :], start=(k == 0),
                                         stop=(k == JC - 1))
                    nc.scalar.copy(vj[:, tkc, :], ps_[:])

                for s in range(BL):
                    attOj = sp.tile([128, JC, L], F32R, tag="attOj", bufs=2)
                    for h in range(H):
                        hc, hp = JPIECE[h]
                        expS = sp.tile([128, 2, L], F32R, tag="expSa", bufs=2)
                        for kc in range(2):
                            ps_s = pp.tile([128, L], F32, tag="mm")
                            nc.tensor.matmul(
                                ps_s[:],
                                kj[hp:hp + 64, hc, s * L + kc * 128:s * L + (kc + 1) * 128],
                                qj[hp:hp + 64, hc, s * L:(s + 1) * L],
                                start=True, stop=True)
                            nc.scalar.activation(expS[:, kc, :], ps_s[:], AF.Exp)
                        ps_cs = pq.tile([1, L], F32, tag="stat")
                        for kc in range(2):
                            nc.tensor.matmul(ps_cs[:], ones[:, 0:1], expS[:, kc, :],
                                             start=(kc == 0), stop=(kc == 1))
                        rec = sp.tile([1, L], F32R, tag="reca", bufs=2)
                        with nc.allow_low_precision(reason="softmax recip"):
                            nc.vector.reciprocal(rec[:], ps_cs[:])
                        ps_bc = pq.tile([128, L], F32, tag="bc")
                        nc.tensor.matmul(ps_bc[:], ones[0:1, :], rec[:],
                                         start=True, stop=True)
                        for kc in range(2):
                            nc.vector.tensor_tensor(expS[:, kc, :], expS[:, kc, :],
                                                    ps_bc[:], ALU.mult)
                        ps_av = pp.tile([128, L], F32, tag="mm")
                        for kc in range(2):
                            nc.tensor.matmul(ps_av[0:64, :],
                                             vj[:, s * 2 + kc, h * 64:(h + 1) * 64],
                                             expS[:, kc, :],
                                             start=(kc == 0), stop=(kc == 1))
                        nc.scalar.copy(attOj[hp:hp + 64, hc, :], ps_av[0:64, :])
                    with nc.allow_low_precision(reason="attn q-sum"):
                        nc.vector.reduce_sum(attm_a[:, :, s:s + 1], attOj[:],
                                             axis=AX.X)
                # m_t2i = Wo @ attm/256 + bo -> add x1a -> z1
                for m in range(JC):
                    ps_ = pp.tile([128, BL], F32, tag="mm")
                    for k in range(JC):
                        nc.tensor.matmul(ps_[:], woa[:, k, m * 128:(m + 1) * 128],
                                         attm_a[:, k, :], start=(k == 0),
                                         stop=(k == JC - 1))
                    tmp_m = sp.tile([128, BL], F32, tag="tmpm", bufs=2)
                    nc.scalar.activation(tmp_m[:], ps_[:], AF.Identity,
                                         bias=boa[:, m:m + 1], scale=1.0 / L)
                    nc.vector.tensor_tensor(z1[:, m, :], tmp_m[:], x1a[:, m, :],
                                            ALU.add)

            # ---------------- stage E2: i2t cross attention (full) + patches
            with tc.tile_pool(name="w_b", bufs=1) as wp, \
                 tc.tile_pool(name="s_b", bufs=1) as sp, \
                 tc.tile_pool(name="pp_b", bufs=3, space="PSUM") as pp:
                jp = jnt["b"]
                wqb = wp.tile([128, JC, J], F32R)
                nc.sync.dma_start(wqb[:], jp["wq"].ap().rearrange("k p m -> p k m"))
                wkb = wp.tile([128, JC, J], F32R)
                nc.sync.dma_start(wkb[:], jp["wk"].ap().rearrange("k p m -> p k m"))
                wvb = wp.tile([128, JC, J], F32R)
                nc.sync.dma_start(wvb[:], jp["wv"].ap().rearrange("k p m -> p k m"))
                wob = wp.tile([128, JC, J], F32R)
                nc.sync.dma_start(wob[:], jp["wo"].ap().rearrange("k p m -> p k m"))
                bqb = wp.tile([128, JC], F32); nc.sync.dma_start(bqb[:], jp["bq"].ap())
                bob = wp.tile([128, JC], F32); nc.sync.dma_start(bob[:], jp["bo"].ap())
                qin2 = sp.tile([128, JC, T], F32R, tag="qin2")
                nc.sync.dma_start(qin2[:], dr["QI"].ap().rearrange("k p t -> p k t"))
                kin2 = sp.tile([128, JC, T], F32R, tag="kin2")
                nc.sync.dma_start(kin2[:], dr["KT"].ap().rearrange("k p t -> p k t"))
                vin2 = sp.tile([128, JC, T], F32R, tag="vin2")
                nc.sync.dma_start(vin2[:], dr["VT"].ap().rearrange("k p t -> p k t"))

                qj2 = sp.tile([128, JC, T], F32R, tag="qj2")
                kj2 = sp.tile([128, JC, T], F32R, tag="kj2")
                for m in range(JC):
                    for t4 in range(4):
                        tsl = slice(t4 * 512, (t4 + 1) * 512)
                        ps_ = pp.tile([128, 512], F32, tag="mm")
                        for k in range(JC):
                            nc.tensor.matmul(ps_[:], wqb[:, k, m * 128:(m + 1) * 128],
                                             qin2[:, k, tsl], start=(k == 0),
                                             stop=(k == JC - 1))
                        nc.scalar.activation(qj2[:, m, tsl], ps_[:], AF.Identity,
                                             bias=bqb[:, m:m + 1])
                        ps_ = pp.tile([128, 512], F32, tag="mm")
                        for k in range(JC):
                            nc.tensor.matmul(ps_[:], wkb[:, k, m * 128:(m + 1) * 128],
                                             kin2[:, k, tsl], start=(k == 0),
                                             stop=(k == JC - 1))
                        nc.scalar.copy(kj2[:, m, tsl], ps_[:])
                vj2 = sp.tile([128, 2 * BL, J], F32R, tag="vj2")
                for tkc in range(2 * BL):
                    ps_ = pp.tile([128, J], F32, tag="mm")
                    for k in range(JC):
                        nc.tensor.matmul(ps_[:], vin2[:, k, tkc * 128:(tkc + 1) * 128],
                                         wvb[:, k, :], start=(k == 0),
                                         stop=(k == JC - 1))
                    nc.scalar.copy(vj2[:, tkc, :], ps_[:])

                atb = sp.tile([128, JC, T], F32R, tag="atb")  # full i2t output
                for s in range(BL):
                    attOj = sp.tile([128, JC, L], F32R, tag="attOj2", bufs=2)
                    for h in range(H):
                        hc, hp = JPIECE[h]
                        expS = sp.tile([128, 2, L], F32R, tag="expSb", bufs=2)
                        for kc in range(2):
                            ps_s = pp.tile([128, L], F32, tag="mm")
                            nc.tensor.matmul(
                                ps_s[:],
                                kj2[hp:hp + 64, hc, s * L + kc * 128:s * L + (kc + 1) * 128],
                                qj2[hp:hp + 64, hc, s * L:(s + 1) * L],
                                start=True, stop=True)
                            nc.scalar.activation(expS[:, kc, :], ps_s[:], AF.Exp)
                        ps_cs = pq.tile([1, L], F32, tag="stat")
                        for kc in range(2):
                            nc.tensor.matmul(ps_cs[:], ones[:, 0:1], expS[:, kc, :],
                                             start=(kc == 0), stop=(kc == 1))
                        rec = sp.tile([1, L], F32R, tag="recb", bufs=2)
                        with nc.allow_low_precision(reason="softmax recip"):
                            nc.vector.reciprocal(rec[:], ps_cs[:])
                        ps_bc = pq.tile([128, L], F32, tag="bc")
                        nc.tensor.matmul(ps_bc[:], ones[0:1, :], rec[:],
                                         start=True, stop=True)
                        for kc in range(2):
                            nc.vector.tensor_tensor(expS[:, kc, :], expS[:, kc, :],
                                                    ps_bc[:], ALU.mult)
                        ps_av = pp.tile([128, L], F32, tag="mm")
                        for kc in range(2):
                            nc.tensor.matmul(ps_av[0:64, :],
                                             vj2[:, s * 2 + kc, h * 64:(h + 1) * 64],
                                             expS[:, kc, :],
                                             start=(kc == 0), stop=(kc == 1))
                        nc.scalar.copy(attOj[hp:hp + 64, hc, :], ps_av[0:64, :])
                    # o-projection per sample, accumulate q-sums via accum_out
                    for m in range(JC):
                        ps_ = pp.tile([128, L], F32, tag="mm")
                        for k in range(JC):
                            nc.tensor.matmul(ps_[:], wob[:, k, m * 128:(m + 1) * 128],
                                             attOj[:, k, :], start=(k == 0),
                                             stop=(k == JC - 1))
                        with nc.allow_low_precision(reason="i2t q-sum"):
                            nc.scalar.activation(atb[:, m, s * L:(s + 1) * L], ps_[:],
                                                 AF.Identity, bias=bob[:, m:m + 1],
                                                 accum_out=attm_b[:, m, s:s + 1])
                # z2 = tcls + attm_b/256
                for m in range(JC):
                    nc.vector.scalar_tensor_tensor(z2[:, m, :], attm_b[:, m, :],
                                                   1.0 / L, tcls[:, m, :],
                                                   ALU.mult, ALU.add)

                # final LNs (over 256 features)
                with tc.tile_pool(name="s_f", bufs=1) as sf:
                    ln1gt = sf.tile([128, JC], F32); nc.sync.dma_start(ln1gt[:], ln1g.ap())
                    ln1bt = sf.tile([128, JC], F32); nc.sync.dma_start(ln1bt[:], ln1b.ap())
                    ln2gt = sf.tile([128, JC], F32); nc.sync.dma_start(ln2gt[:], ln2g.ap())
                    ln2bt = sf.tile([128, JC], F32); nc.sync.dma_start(ln2bt[:], ln2b.ap())
                    _ln_partition(nc, sf, pq, z1, JC, BL, ones, ln1gt, ln1bt, x1t,
                                  tag="l1", epst=epst, zerot=zerot)
                    _ln_partition(nc, sf, pq, z2, JC, BL, ones, ln2gt, ln2bt, x2t,
                                  tag="l2", epst=epst, zerot=zerot)

                # ---------------- output assembly (transpose to token-major)
                with tc.tile_pool(name="s_o", bufs=3) as so, \
                     tc.tile_pool(name="pp_o", bufs=1, space="PSUM") as po:
                    ppatch = sp.tile([128, JC, T], F32R, tag="ppt")
                    nc.sync.dma_start(ppatch[:],
                                      dr["PP"].ap().rearrange("k p t -> p k t"))
                    fusec = sp.tile([128, JC, T], F32R, tag="fusec")
                    nc.vector.tensor_tensor(fusec[:], ppatch[:], atb[:], ALU.add)
                    for tkc in range(2 * BL):
                        s = tkc // 2
                        row0 = 1 + (tkc % 2) * 128
                        stg = so.tile([128, J], F32, tag="stg")
                        for c in range(JC):
                            pt = po.tile([128, 128], F32R, tag="tp")
                            nc.tensor.transpose(
                                pt[:], fusec[:, c, tkc * 128:(tkc + 1) * 128],
                                identr[:])
                            nc.scalar.copy(stg[:, c * 128:(c + 1) * 128], pt[:])
                        nc.sync.dma_start(out_d[s, row0:row0 + 128, :], stg[:])
                    x1s = so.tile([BL, J], F32, tag="xs")
                    x2s = so.tile([BL, J], F32, tag="xs")
                    for c in range(JC):
                        pt = po.tile([BL, 128], F32R, tag="tp")
                        nc.tensor.transpose(pt[:], x1t[:, c, :], identr[:])
                        nc.scalar.copy(x1s[:, c * 128:(c + 1) * 128], pt[:])
                        pt = po.tile([BL, 128], F32R, tag="tp")
                        nc.tensor.transpose(pt[:], x2t[:, c, :], identr[:])
                        nc.scalar.copy(x2s[:, c * 128:(c + 1) * 128], pt[:])
                    nc.sync.dma_start(out_d[:, 0, :], x1s[:])
                    nc.sync.dma_start(out_d[:, 1 + NPATCH, :], x2s[:])

    nc.compile()
    return nc


def _pack_params(params):
    """Host-side packing + algebraic folds of the parameter pytree."""
    def A(x):
        return np.asarray(x, dtype=np.float32)

    out = {}
    s192 = 1.0 / math.sqrt(DH)
    alphas = []
    for pre, key in (("t", "txt_enh"), ("i", "ip_enh")):
        e = params[key]
        m = e["mha"]
        out[f"{pre}_wq"] = _pack_wT(A(m["q"]["W"]) * s192)
        out[f"{pre}_bq"] = _pack_b(A(m["q"]["b"]) * s192)
        out[f"{pre}_wk"] = _pack_wT(A(m["k"]["W"]))
        out[f"{pre}_wv"] = _pack_wT(A(m["v"]["W"]))
        out[f"{pre}_wo"] = _pack_wT(A(m["o"]["W"]))
        out[f"{pre}_bo"] = _pack_b(A(m["o"]["W"]) @ A(m["v"]["b"]) + A(m["o"]["b"]))
        pos = A(e["pos"])[0, :L]                               # [L, D]
        out[f"{pre}_pos"] = np.ascontiguousarray(pos.T.reshape(DC, 128, L))
        out[f"{pre}_lng"] = _pack_b(A(e["ln"]["g"]))
        out[f"{pre}_lnb"] = _pack_b(A(e["ln"]["b"]))
        alphas.append(float(A(e["alpha"]).reshape(-1)[0]))
    g = params["ig_enh"]
    gm = g["mha"]
    out["g_w"] = _pack_wT(A(gm["o"]["W"]) @ A(gm["v"]["W"]))
    out["g_bo"] = _pack_b(A(gm["o"]["W"]) @ A(gm["v"]["b"]) + A(gm["o"]["b"]))
    gpos = A(g["pos"])[0, 0]                                   # [D]
    out["g_pos"] = np.ascontiguousarray(
        np.repeat(gpos.reshape(DC, 128, 1), BL, axis=2))
    out["g_lng"] = _pack_b(A(g["ln"]["g"]))
    out["g_lnb"] = _pack_b(A(g["ln"]["b"]))
    alphas.append(float(A(g["alpha"]).reshape(-1)[0]))

    out["c_qt"] = _pack_wT(A(params["query_txt"]["W"]))
    out["c_kt"] = _pack_wT(A(params["key_txt"]["W"]))
    out["c_vt"] = _pack_wT(A(params["value_txt"]["W"]))
    out["c_ki"] = _pack_wT(A(params["key_img"]["W"]))
    out["c_vi"] = _pack_wT(A(params["value_img"]["W"]))
    out["c_qi"] = _pack_wT(A(params["query_img"]["W"]))
    out["c_pp"] = _pack_wT(A(params["img_patch_proj"]["W"]))
    out["c_pp_b"] = _pack_b(A(params["img_patch_proj"]["b"]))
    out["c_gp"] = _pack_wT(A(params["img_global_proj"]["W"]))
    out["c_gp_b"] = _pack_b(A(params["img_global_proj"]["b"]))
    out["c_tp"] = _pack_wT(A(params["txt_proj"]["W"]))
    out["c_tp_b"] = _pack_b(A(params["txt_proj"]["b"]))

    s64 = 1.0 / math.sqrt(J // H)
    for pre, key, qb_key, vb_key in (("a", "attn_t2i", "query_txt", "value_img"),
                                     ("b", "attn_i2t", "query_img", "value_txt")):
        m = params[key]
        out[f"{pre}_wq"] = _pack_wT(A(m["q"]["W"]) * s64)
        out[f"{pre}_bq"] = _pack_b(
            (A(m["q"]["W"]) @ A(params[qb_key]["b"]) + A(m["q"]["b"])) * s64)
        out[f"{pre}_wk"] = _pack_wT(A(m["k"]["W"]))
        out[f"{pre}_wv"] = _pack_wT(A(m["v"]["W"]))
        out[f"{pre}_wo"] = _pack_wT(A(m["o"]["W"]))
        out[f"{pre}_bo"] = _pack_b(
            A(m["o"]["W"]) @ (A(m["v"]["W"]) @ A(params[vb_key]["b"])
                              + A(m["v"]["b"])) + A(m["o"]["b"]))
    out["ln1g"] = _pack_b(A(params["ln_img"]["g"]))
    out["ln1b"] = _pack_b(A(params["ln_img"]["b"]))
    out["ln2g"] = _pack_b(A(params["ln_txt"]["g"]))
    out["ln2b"] = _pack_b(A(params["ln_txt"]["b"]))
    out["ones"] = np.ones((128, 128), np.float32)
    out["consts"] = np.array([[EPS, 0.0]], np.float32)
    out["identr"] = np.eye(128, dtype=np.float32)
    return out, tuple(alphas)


def kernel(img_global, img_patch, txt_feats, params):
    img_global = np.asarray(img_global, np.float32)
    img_patch = np.asarray(img_patch, np.float32)
    txt_feats = np.asarray(txt_feats, np.float32)
    packed, alphas = _pack_params(params)

    if alphas not in _prog_cache:
        _prog_cache[alphas] = _build_program(*alphas)
    nc = _prog_cache[alphas]

    in_maps = []
    for c in range(N_CORES):
        sl = slice(c * BL, (c + 1) * BL)
        m = dict(packed)
        m["xt"] = _pack_xT(txt_feats[sl])
        m["xi"] = _pack_xT(img_patch[sl])
        m["xg"] = np.ascontiguousarray(
            img_global[sl].T.reshape(DC, 128, BL))
        in_maps.append(m)

    import os
    kwargs = {}
    if os.environ.get("BASS_KERNEL_TRACE"):
        # dev-only: register the axon NTFF profiling hook so trace=True works
        import types
        import antenv
        if "antenv.axon_hooks" not in sys.modules:
            _m = types.ModuleType("antenv.axon_hooks")
            _h = [None]
            _m.set_axon_ntff_profile_hook = lambda h: _h.__setitem__(0, h)
            _m.get_axon_ntff_profile_hook = lambda: _h[0]
            sys.modules["antenv.axon_hooks"] = _m
            antenv.axon_hooks = _m
            from trn_agent_boot.trn_boot import _ntff_profile_via_ctypes
            _m.set_axon_ntff_profile_hook(
                _ntff_profile_via_ctypes("/opt/axon/libaxon_pjrt.so"))
        kwargs = dict(trace=True, tmpdir=os.environ["BASS_KERNEL_TRACE"])

    res = run_bass_kernel_spmd(nc, in_maps, list(range(N_CORES)), **kwargs)
    kernel.last_results = res
    out = np.concatenate([res.results[c]["out"] for c in range(N_CORES)], axis=0)
    return np.ascontiguousarray(out, dtype=np.float32)


kernel.last_results = None


# revision 14
# speedup vs baseline: 1.3872x; 1.3872x over previous
"""Trainium2 Bass kernel for nn_CrossModalFusion (dense cross-modal transformer).

Contract: kernel(**inputs) takes FULL inputs (B=64), shards batch across 8
NeuronCores (pure data parallel, 8 samples/core), runs one SPMD Bass program,
gathers to the full (64, 258, 256) output.

Layout: activations feature-major (features on SBUF partitions, tokens free) so
every linear contracts over partitions. fp32r operands everywhere (full PE rate,
~1e-4 rounding). LayerNorm gains are folded into downstream projection weights;
the per-token shift enters those projections as a rank-1 K=1 matmul term, so the
normalized activation tensor is never materialized for the big enhancers.
"""

import sys

sys.path.insert(0, "/opt/trn_rl_repo")

import math
import numpy as np

import concourse.bass as bass  # noqa: F401
import concourse.bacc as bacc
import concourse.tile as tile
from concourse import mybir
from concourse.bass_utils import run_bass_kernel_spmd

F32 = mybir.dt.float32
F32R = mybir.dt.float32r
I32 = mybir.dt.int32
AF = mybir.ActivationFunctionType
ALU = mybir.AluOpType
AX = mybir.AxisListType

N_CORES = 8
B = 64
BL = B // N_CORES          # 8 samples per core
NPATCH = 256
L = 256                    # tokens per sample (both modalities)
T = BL * L                 # 2048 tokens per core
D = 768
DC = D // 128              # 6 feature chunks
J = 256
JC = J // 128              # 2 joint chunks
H = 4
DH = D // H                # 192
EPS = 1e-5
RSQRT_MAGIC = 0x5f3759df

# (chunk, partition_offset, size) pieces of each 192-row head in the 6x128 grid
PIECES = {
    0: [(0, 0, 128), (1, 0, 64)],
    1: [(1, 64, 64), (2, 0, 128)],
    2: [(3, 0, 128), (4, 0, 64)],
    3: [(4, 64, 64), (5, 0, 128)],
}
JPIECE = {h: (h // 2, (h % 2) * 64) for h in range(H)}

_prog_cache = {}


def _pack_wT(w):
    # w [dout, din] -> [din/128, 128, dout]
    dout, din = w.shape
    return np.ascontiguousarray(w.T.reshape(din // 128, 128, dout), dtype=np.float32)


def _pack_b(b):
    # b [dout] -> [128, dout/128]
    dout = b.shape[0]
    return np.ascontiguousarray(b.reshape(dout // 128, 128).T, dtype=np.float32)


def _pack_xT(x):
    # x [BL, L, D] -> [DC, 128, BL*L]
    bl, l, d = x.shape
    return np.ascontiguousarray(x.reshape(bl * l, d).T.reshape(d // 128, 128, bl * l),
                                dtype=np.float32)


def _rsqrt_row(nc, sp, vpe, ncols, cmagic, tag):
    """DVE-only Newton rsqrt of a [1, ncols] f32 AP (var+eps). Returns an
    f32r [1, ncols] tile. No ACT involvement (avoids table-set switches)."""
    sh = sp.tile([1, ncols], I32, tag=tag + "_sh")
    nc.vector.tensor_scalar(sh[:], vpe.bitcast(I32), 1, None,
                            ALU.arith_shift_right)
    y0 = sp.tile([1, ncols], F32, tag=tag + "_y0")
    nc.vector.tensor_tensor(y0[:].bitcast(I32), cmagic[0:1, 0:ncols], sh[:],
                            ALU.subtract)
    a = sp.tile([1, ncols], F32, tag=tag + "_a")
    c = sp.tile([1, ncols], F32, tag=tag + "_c")
    y1 = sp.tile([1, ncols], F32, tag=tag + "_y1")
    out = sp.tile([1, ncols], F32R, tag=tag + "_rstd")
    for it in range(2):
        src = y0 if it == 0 else y1
        nc.vector.tensor_tensor(a[:], src[:], src[:], ALU.mult)
        nc.vector.tensor_tensor(a[:], a[:], vpe, ALU.mult)
        nc.vector.tensor_scalar(c[:], a[:], -0.5, 1.5, ALU.mult, ALU.add)
        if it == 0:
            nc.vector.tensor_tensor(y1[:], y0[:], c[:], ALU.mult)
        else:
            with nc.allow_low_precision(reason="ln rstd"):
                nc.vector.tensor_tensor(out[:], y1[:], c[:], ALU.mult)
    return out


def _ln_stats(nc, sp, pq, r, nch, ncols, ones, cmagic, tag):
    """Partition-dim LN stats of F-major r [128, nch, ncols].
    Returns (rstd [1,ncols] f32r tile, b2 = -mean*rstd [1,ncols] f32r tile)."""
    nfeat = float(nch * 128)
    sq = sp.tile([128, nch, ncols], F32R, tag=tag + "_sq")
    nc.vector.tensor_tensor(sq[:], r[:], r[:], ALU.mult)
    ps_sum = pq.tile([1, ncols], F32, tag="stat")
    for k in range(nch):
        nc.tensor.matmul(ps_sum[:], ones[:, 0:1], r[:, k, :],
                         start=(k == 0), stop=(k == nch - 1))
    ps_ssq = pq.tile([1, ncols], F32, tag="stat")
    for k in range(nch):
        nc.tensor.matmul(ps_ssq[:], ones[:, 0:1], sq[:, k, :],
                         start=(k == 0), stop=(k == nch - 1))
    mean = sp.tile([1, ncols], F32, tag=tag + "_mean")
    nc.vector.tensor_scalar_mul(mean[:], ps_sum[:], 1.0 / nfeat)
    # msq = mean*mean - eps  (so vpe = ssq/nf - msq = var + eps)
    msq = sp.tile([1, ncols], F32, tag=tag + "_msq")
    nc.vector.tensor_tensor(msq[:], mean[:], mean[:], ALU.mult)
    nc.vector.tensor_scalar(msq[:], msq[:], float(EPS), None, ALU.subtract)
    vpe = sp.tile([1, ncols], F32, tag=tag + "_vpe")
    nc.vector.scalar_tensor_tensor(vpe[:], ps_ssq[:], 1.0 / nfeat, msq[:],
                                   ALU.mult, ALU.subtract)
    rstd = _rsqrt_row(nc, sp, vpe[:], ncols, cmagic, tag)
    b2 = sp.tile([1, ncols], F32R, tag=tag + "_b2")
    with nc.allow_low_precision(reason="ln b2"):
        nc.vector.scalar_tensor_tensor(b2[:], mean[:], -1.0, rstd[:],
                                       ALU.mult, ALU.mult)
    return rstd, b2


def _softmax_attn(nc, sp, pp, pq, ones, attO, pieces_of, kslice_of, vcol_of,
                  tagpre):
    """Head-interleaved softmax attention for one sample.

    attO: output [128, nch, L] f32r tile, filled piecewise (unnormalized
    exp-scores are normalized via a broadcast multiply before the AV matmuls).
    kslice_of(h, kc) -> list of (lhsT, rhs) matmul operand pairs for S_T.
    vcol_of(kc, c, po, sz) -> token-major V columns AP for the AV lhsT.
    """
    expS = {}
    ps_s = {}
    for h in range(H):
        ps_s[h] = pp.tile([128, 2 * L], F32, tag="st", bufs=2, name=f"ps_s{h}")
        for kc in range(2):
            pcs = kslice_of(h, kc)
            for pi, (lhs, rhs) in enumerate(pcs):
                nc.tensor.matmul(ps_s[h][:, kc * L:(kc + 1) * L], lhs, rhs,
                                 start=(pi == 0), stop=(pi == len(pcs) - 1))
    for h in range(H):
        e = sp.tile([128, 2, L], F32R, tag=tagpre + "expS", bufs=2,
                    name=f"expS{h}")
        for kc in range(2):
            nc.scalar.activation(e[:, kc, :], ps_s[h][:, kc * L:(kc + 1) * L],
                                 AF.Exp)
        expS[h] = e
    recs = {}
    for h in range(H):
        ps_cs = pq.tile([1, L], F32, tag="stat")
        for kc in range(2):
            nc.tensor.matmul(ps_cs[:], ones[:, 0:1], expS[h][:, kc, :],
                             start=(kc == 0), stop=(kc == 1))
        rec_f = sp.tile([1, L], F32, tag=tagpre + "recf", bufs=2,
                        name=f"recf{h}")
        nc.vector.reciprocal_approx_fast(rec_f[:], ps_cs[:])
        rec_r = sp.tile([1, L], F32R, tag=tagpre + "recr", bufs=2,
                        name=f"recr{h}")
        nc.vector.tensor_copy(rec_r[:], rec_f[:])
        recs[h] = rec_r
    bcs = {}
    for h in range(H):
        ps_bc = pq.tile([128, L], F32, tag="bc")
        nc.tensor.matmul(ps_bc[:], ones[0:1, :], recs[h][:], start=True,
                         stop=True)
        bcs[h] = ps_bc
    for h in range(H):
        for kc in range(2):
            nc.vector.tensor_tensor(expS[h][:, kc, :], expS[h][:, kc, :],
                                    bcs[h][:], ALU.mult)
    for h in range(H):
        for (c, po, sz) in pieces_of(h):
            ps_av = pp.tile([128, L], F32, tag="mm")
            for kc in range(2):
                nc.tensor.matmul(ps_av[0:sz, :], vcol_of(kc, c, po, sz),
                                 expS[h][:, kc, :], start=(kc == 0),
                                 stop=(kc == 1))
            nc.scalar.copy(attO[po:po + sz, c, :], ps_av[0:sz, :])


def _build_program(alpha_txt, alpha_ip, alpha_ig):
    nc = bacc.Bacc("TRN2", target_bir_lowering=False, debug=False,
                   num_devices=N_CORES)

    def din(name, shape, dt=F32R):
        return nc.dram_tensor(name, shape, dt, kind="ExternalInput")

    xt = din("xt", [DC, 128, T])
    xi = din("xi", [DC, 128, T])
    xg = din("xg", [DC, 128, BL])
    enh_p = {}
    for pre in ("t", "i"):
        enh_p[pre] = dict(
            wq=din(f"{pre}_wq", [DC, 128, D]), wk=din(f"{pre}_wk", [DC, 128, D]),
            wv=din(f"{pre}_wv", [DC, 128, D]), wo=din(f"{pre}_wo", [DC, 128, D]),
            bq=din(f"{pre}_bq", [128, DC], F32), bo=din(f"{pre}_bo", [128, DC], F32),
            pos=din(f"{pre}_pos", [DC, 128, L]),
        )
    g_w = din("g_w", [DC, 128, D])
    g_bo = din("g_bo", [128, DC], F32)
    g_pos = din("g_pos", [DC, 128, BL])
    cr_names = ("c_qt", "c_kt", "c_vt", "c_ki", "c_vi", "c_qi", "c_pp",
                "c_gp", "c_tp")
    cr_w = {n: din(n, [DC, 128, J]) for n in cr_names}
    cr_u = {n: din(n + "_u", [1, J]) for n in cr_names}
    c_pp_b = din("c_pp_b", [128, JC], F32)
    c_gp_b = din("c_gp_b", [128, JC], F32)
    c_tp_b = din("c_tp_b", [128, JC], F32)
    jnt = {}
    for pre in ("a", "b"):  # a = t2i, b = i2t
        jnt[pre] = dict(
            wq=din(f"{pre}_wq", [JC, 128, J]), wk=din(f"{pre}_wk", [JC, 128, J]),
            wv=din(f"{pre}_wv", [JC, 128, J]), wo=din(f"{pre}_wo", [JC, 128, J]),
            bq=din(f"{pre}_bq", [128, JC], F32), bo=din(f"{pre}_bo", [128, JC], F32),
        )
    ln1g = din("ln1g", [128, JC], F32); ln1b = din("ln1b", [128, JC], F32)
    ln2g = din("ln2g", [128, JC], F32); ln2b = din("ln2b", [128, JC], F32)
    ones_d = din("ones", [128, 128])
    identr_d = din("identr", [128, 128])

    dr = {nm: nc.dram_tensor(nm, [JC, 128, T], F32R)
          for nm in ("QT", "KT", "VT", "KI", "VI", "QI", "PP")}
    out_d = nc.dram_tensor("out", [BL, 1 + NPATCH + 1, J], F32,
                           kind="ExternalOutput")

    with tile.TileContext(nc) as tc:
        with tc.tile_pool(name="glob", bufs=1) as gp, \
             tc.tile_pool(name="gps", bufs=2, space="PSUM") as pq:
            ones = gp.tile([128, 128], F32R)
            nc.sync.dma_start(ones[:], ones_d[:])
            identr = gp.tile([128, 128], F32R)
            nc.sync.dma_start(identr[:], identr_d[:])
            cmagic = gp.tile([1, L], I32)
            nc.vector.memset(cmagic[:], RSQRT_MAGIC)
            rm_first = gp.tile([128, DC, BL], F32R)   # txt rM first tokens
            b2_first = gp.tile([1, BL], F32R)         # txt B2 first tokens
            tcls = gp.tile([128, JC, BL], F32R)
            x1a = gp.tile([128, JC, BL], F32R)
            attm_a = gp.tile([128, JC, BL], F32R)
            attm_b = gp.tile([128, JC, BL], F32R)
            z1 = gp.tile([128, JC, BL], F32R)
            z2 = gp.tile([128, JC, BL], F32R)
            x1t = gp.tile([128, JC, BL], F32R)
            x2t = gp.tile([128, JC, BL], F32R)

            # ---------------- stage A/B: 768-dim self-attn enhancers
            for pre, src, alpha, crlist in (
                ("t", xt, alpha_txt, [("c_qt", dr["QT"]), ("c_kt", dr["KT"]),
                                      ("c_vt", dr["VT"])]),
                ("i", xi, alpha_ip, [("c_ki", dr["KI"]), ("c_vi", dr["VI"]),
                                     ("c_qi", dr["QI"]), ("c_pp", dr["PP"])]),
            ):
                p = enh_p[pre]
                with tc.tile_pool(name=f"w_{pre}", bufs=1) as wp, \
                     tc.tile_pool(name=f"s_{pre}", bufs=1) as sp, \
                     tc.tile_pool(name=f"pp_{pre}", bufs=2, space="PSUM") as pp:
                    wq = wp.tile([128, DC, D], F32R)
                    nc.sync.dma_start(wq[:], p["wq"].ap().rearrange("k p m -> p k m"))
                    wk = wp.tile([128, DC, D], F32R)
                    nc.sync.dma_start(wk[:], p["wk"].ap().rearrange("k p m -> p k m"))
                    wv = wp.tile([128, DC, D], F32R)
                    nc.sync.dma_start(wv[:], p["wv"].ap().rearrange("k p m -> p k m"))
                    wo = wp.tile([128, DC, D], F32R)
                    nc.sync.dma_start(wo[:], p["wo"].ap().rearrange("k p m -> p k m"))
                    pos = wp.tile([128, DC, L], F32R)
                    nc.sync.dma_start(pos[:], p["pos"].ap().rearrange("k p t -> p k t"))
                    bq = wp.tile([128, DC], F32); nc.sync.dma_start(bq[:], p["bq"].ap())
                    bo = wp.tile([128, DC], F32); nc.sync.dma_start(bo[:], p["bo"].ap())
                    cw, cu = [], []
                    for ci, (cn, _) in enumerate(crlist):
                        t_ = wp.tile([128, DC, J], F32R, tag=f"cw{ci}")
                        nc.sync.dma_start(t_[:], cr_w[cn].ap().rearrange("k p m -> p k m"))
                        cw.append(t_)
                        u_ = wp.tile([1, J], F32R, tag=f"cu{ci}")
                        nc.sync.dma_start(u_[:], cr_u[cn].ap())
                        cu.append(u_)
                    ppb = None
                    if pre == "i":
                        ppb = wp.tile([128, JC], F32)
                        nc.sync.dma_start(ppb[:], c_pp_b.ap())

                    xs, qs, ks, vs = {}, {}, {}, {}

                    def proj_sample(s, src=src, wq=wq, wk=wk, wv=wv, pos=pos,
                                    bq=bq, sp=sp, pp=pp, xs=xs, qs=qs, ks=ks,
                                    vs=vs):
                        sl = slice(s * L, (s + 1) * L)
                        x = sp.tile([128, DC, L], F32R, tag="x", bufs=2,
                                    name=f"x{s}")
                        nc.sync.dma_start(
                            x[:], src[:, :, sl].rearrange("k p t -> p k t"))
                        nc.vector.tensor_tensor(x[:], x[:], pos[:], ALU.add)
                        q = sp.tile([128, DC, L], F32R, tag="q", bufs=2,
                                    name=f"q{s}")
                        for m in range(DC):
                            ps_ = pp.tile([128, L], F32, tag="mm")
                            for k in range(DC):
                                nc.tensor.matmul(
                                    ps_[:], wq[:, k, m * 128:(m + 1) * 128],
                                    x[:, k, :], start=(k == 0), stop=(k == DC - 1))
                            nc.scalar.activation(q[:, m, :], ps_[:], AF.Identity,
                                                 bias=bq[:, m:m + 1])
                        kt = sp.tile([128, DC, L], F32R, tag="k", bufs=2,
                                     name=f"kt{s}")
                        for m in range(DC):
                            ps_ = pp.tile([128, L], F32, tag="mm")
                            for k in range(DC):
                                nc.tensor.matmul(
                                    ps_[:], wk[:, k, m * 128:(m + 1) * 128],
                                    x[:, k, :], start=(k == 0), stop=(k == DC - 1))
                            nc.scalar.copy(kt[:, m, :], ps_[:])
                        v = sp.tile([128, 2, D], F32R, tag="v", bufs=1,
                                    name=f"v{s}")
                        for tkc in range(2):
                            for dsl in range(2):
                                ps_ = pp.tile([128, 384], F32, tag="mm")
                                for k in range(DC):
                                    nc.tensor.matmul(
                                        ps_[:], x[:, k, tkc * 128:(tkc + 1) * 128],
                                        wv[:, k, dsl * 384:(dsl + 1) * 384],
                                        start=(k == 0), stop=(k == DC - 1))
                                nc.scalar.copy(v[:, tkc, dsl * 384:(dsl + 1) * 384],
                                               ps_[:])
                        xs[s], qs[s], ks[s], vs[s] = x, q, kt, v

                    proj_sample(0)
                    for s in range(BL):
                        x, q, kt, v = xs[s], qs[s], ks[s], vs[s]
                        attO = sp.tile([128, DC, L], F32R, tag="scr6", bufs=2,
                                       name=f"attO{s}")

                        def kslice(h, kc, kt=kt, q=q):
                            return [
                                (kt[po:po + sz, c, kc * 128:(kc + 1) * 128],
                                 q[po:po + sz, c, :])
                                for (c, po, sz) in PIECES[h]]

                        def vcol(kc, c, po, sz, v=v):
                            return v[:, kc, c * 128 + po:c * 128 + po + sz]

                        _softmax_attn(nc, sp, pp, pq, ones, attO,
                                      lambda h: PIECES[h], kslice, vcol, "e")

                        r = sp.tile([128, DC, L], F32R, tag="r", bufs=1,
                                    name=f"r{s}")
                        for m in range(DC):
                            ps_ = pp.tile([128, L], F32, tag="mm")
                            for k in range(DC):
                                nc.tensor.matmul(
                                    ps_[:], wo[:, k, m * 128:(m + 1) * 128],
                                    attO[:, k, :], start=(k == 0), stop=(k == DC - 1))
                            if alpha != 1.0:
                                nc.vector.tensor_scalar_mul(x[:, m, :], x[:, m, :],
                                                            float(alpha))
                            nc.vector.scalar_tensor_tensor(
                                r[:, m, :], ps_[:], bo[:, m:m + 1], x[:, m, :],
                                ALU.add, ALU.add)

                        rstd, b2 = _ln_stats(nc, sp, pq, r, DC, L, ones, cmagic,
                                             "ln")

                        # emit next sample's projections here so the PE has
                        # independent work during the DVE LN-stat chain
                        if s + 1 < BL:
                            proj_sample(s + 1)

                        ps_m = pq.tile([128, L], F32, tag="bc")
                        nc.tensor.matmul(ps_m[:], ones[0:1, :], rstd[:],
                                         start=True, stop=True)
                        rM = sp.tile([128, DC, L], F32R, tag="scr6b", bufs=1,
                                     name=f"rM{s}")
                        for m in range(DC):
                            nc.vector.tensor_tensor(rM[:, m, :], r[:, m, :],
                                                    ps_m[:], ALU.mult)
                        if pre == "t":
                            nc.vector.tensor_copy(rm_first[:, :, s:s + 1],
                                                  rM[:, :, 0:1])
                            nc.vector.tensor_copy(b2_first[:, s:s + 1],
                                                  b2[:, 0:1])
                        for ci, (cn, cdram) in enumerate(crlist):
                            cg = sp.tile([128, JC, L], F32R, tag=f"cr{ci}",
                                         bufs=1, name=f"cg{ci}_{s}")
                            for m in range(JC):
                                ps_ = pp.tile([128, L], F32, tag="mm")
                                for k in range(DC):
                                    nc.tensor.matmul(
                                        ps_[:], cw[ci][:, k, m * 128:(m + 1) * 128],
                                        rM[:, k, :], start=(k == 0), stop=False)
                                nc.tensor.matmul(
                                    ps_[:], cu[ci][0:1, m * 128:(m + 1) * 128],
                                    b2[:], start=False, stop=True)
                                if pre == "i" and ci == 3:
                                    nc.scalar.activation(cg[:, m, :], ps_[:],
                                                         AF.Identity,
                                                         bias=ppb[:, m:m + 1])
                                else:
                                    nc.scalar.copy(cg[:, m, :], ps_[:])
                            nc.sync.dma_start(
                                cdram[:, :, s * L:(s + 1) * L].rearrange(
                                    "k p t -> p k t"), cg[:])

            # ---------------- stage C: img_global enhancer (L=1) + small projs
            with tc.tile_pool(name="w_g", bufs=1) as wp, \
                 tc.tile_pool(name="s_g", bufs=1) as sp, \
                 tc.tile_pool(name="pp_g", bufs=2, space="PSUM") as pp:
                gw = wp.tile([128, DC, D], F32R)
                nc.sync.dma_start(gw[:], g_w.ap().rearrange("k p m -> p k m"))
                gbo = wp.tile([128, DC], F32); nc.sync.dma_start(gbo[:], g_bo.ap())
                gpos = wp.tile([128, DC, BL], F32R)
                nc.sync.dma_start(gpos[:], g_pos.ap().rearrange("k p t -> p k t"))
                gpw = wp.tile([128, DC, J], F32R)
                nc.sync.dma_start(gpw[:], cr_w["c_gp"].ap().rearrange("k p m -> p k m"))
                gpu = wp.tile([1, J], F32R); nc.sync.dma_start(gpu[:], cr_u["c_gp"].ap())
                gpb = wp.tile([128, JC], F32); nc.sync.dma_start(gpb[:], c_gp_b.ap())
                tpw = wp.tile([128, DC, J], F32R)
                nc.sync.dma_start(tpw[:], cr_w["c_tp"].ap().rearrange("k p m -> p k m"))
                tpu = wp.tile([1, J], F32R); nc.sync.dma_start(tpu[:], cr_u["c_tp"].ap())
                tpb = wp.tile([128, JC], F32); nc.sync.dma_start(tpb[:], c_tp_b.ap())

                xgt = sp.tile([128, DC, BL], F32R, tag="xg")
                nc.sync.dma_start(xgt[:], xg.ap().rearrange("k p t -> p k t"))
                nc.vector.tensor_tensor(xgt[:], xgt[:], gpos[:], ALU.add)
                rg = sp.tile([128, DC, BL], F32R, tag="rg")
                for m in range(DC):
                    ps_ = pp.tile([128, BL], F32, tag="mm")
                    for k in range(DC):
                        nc.tensor.matmul(ps_[:], gw[:, k, m * 128:(m + 1) * 128],
                                         xgt[:, k, :], start=(k == 0),
                                         stop=(k == DC - 1))
                    if alpha_ig != 1.0:
                        nc.vector.tensor_scalar_mul(xgt[:, m, :], xgt[:, m, :],
                                                    float(alpha_ig))
                    nc.vector.scalar_tensor_tensor(
                        rg[:, m, :], ps_[:], gbo[:, m:m + 1], xgt[:, m, :],
                        ALU.add, ALU.add)
                rstdg, b2g = _ln_stats(nc, sp, pq, rg, DC, BL, ones, cmagic,
                                       "lng")
                ps_m = pq.tile([128, BL], F32, tag="bc")
                nc.tensor.matmul(ps_m[:], ones[0:1, :], rstdg[:], start=True,
                                 stop=True)
                rMg = sp.tile([128, DC, BL], F32R, tag="rMg")
                for m in range(DC):
                    nc.vector.tensor_tensor(rMg[:, m, :], rg[:, m, :], ps_m[:],
                                            ALU.mult)
                for m in range(JC):
                    ps_ = pp.tile([128, BL], F32, tag="mm")
                    for k in range(DC):
                        nc.tensor.matmul(ps_[:], gpw[:, k, m * 128:(m + 1) * 128],
                                         rMg[:, k, :], start=(k == 0), stop=False)
                    nc.tensor.matmul(ps_[:], gpu[0:1, m * 128:(m + 1) * 128],
                                     b2g[:], start=False, stop=True)
                    nc.scalar.activation(x1a[:, m, :], ps_[:], AF.Identity,
                                         bias=gpb[:, m:m + 1])
                for m in range(JC):
                    ps_ = pp.tile([128, BL], F32, tag="mm")
                    for k in range(DC):
                        nc.tensor.matmul(ps_[:], tpw[:, k, m * 128:(m + 1) * 128],
                                         rm_first[:, k, :], start=(k == 0),
                                         stop=False)
                    nc.tensor.matmul(ps_[:], tpu[0:1, m * 128:(m + 1) * 128],
                                     b2_first[:], start=False, stop=True)
                    nc.scalar.activation(tcls[:, m, :], ps_[:], AF.Identity,
                                         bias=tpb[:, m:m + 1])

            # ---------------- stage E1: t2i cross attention (mean only)
            with tc.tile_pool(name="w_a", bufs=1) as wp, \
                 tc.tile_pool(name="s_a", bufs=1) as sp, \
                 tc.tile_pool(name="pp_a", bufs=2, space="PSUM") as pp:
                jp = jnt["a"]
                wqa = wp.tile([128, JC, J], F32R)
                nc.sync.dma_start(wqa[:], jp["wq"].ap().rearrange("k p m -> p k m"))
                wka = wp.tile([128, JC, J], F32R)
                nc.sync.dma_start(wka[:], jp["wk"].ap().rearrange("k p m -> p k m"))
                wva = wp.tile([128, JC, J], F32R)
                nc.sync.dma_start(wva[:], jp["wv"].ap().rearrange("k p m -> p k m"))
                woa = wp.tile([128, JC, J], F32R)
                nc.sync.dma_start(woa[:], jp["wo"].ap().rearrange("k p m -> p k m"))
                bqa = wp.tile([128, JC], F32); nc.sync.dma_start(bqa[:], jp["bq"].ap())
                boa = wp.tile([128, JC], F32); nc.sync.dma_start(boa[:], jp["bo"].ap())
                qin = sp.tile([128, JC, T], F32R, tag="qin")
                nc.sync.dma_start(qin[:], dr["QT"].ap().rearrange("k p t -> p k t"))
                kin = sp.tile([128, JC, T], F32R, tag="kin")
                nc.sync.dma_start(kin[:], dr["KI"].ap().rearrange("k p t -> p k t"))
                vin = sp.tile([128, JC, T], F32R, tag="vin")
                nc.sync.dma_start(vin[:], dr["VI"].ap().rearrange("k p t -> p k t"))

                qj = sp.tile([128, JC, T], F32R, tag="qj")
                kj = sp.tile([128, JC, T], F32R, tag="kj")
                for m in range(JC):
                    for t4 in range(4):
                        tsl = slice(t4 * 512, (t4 + 1) * 512)
                        ps_ = pp.tile([128, 512], F32, tag="mm")
                        for k in range(JC):
                            nc.tensor.matmul(ps_[:], wqa[:, k, m * 128:(m + 1) * 128],
                                             qin[:, k, tsl], start=(k == 0),
                                             stop=(k == JC - 1))
                        nc.scalar.activation(qj[:, m, tsl], ps_[:], AF.Identity,
                                             bias=bqa[:, m:m + 1])
                        ps_ = pp.tile([128, 512], F32, tag="mm")
                        for k in range(JC):
                            nc.tensor.matmul(ps_[:], wka[:, k, m * 128:(m + 1) * 128],
                                             kin[:, k, tsl], start=(k == 0),
                                             stop=(k == JC - 1))
                        nc.scalar.copy(kj[:, m, tsl], ps_[:])
                vj = sp.tile([128, 2 * BL, J], F32R, tag="vj")
                for tkc in range(2 * BL):
                    ps_ = pp.tile([128, J], F32, tag="mm")
                    for k in range(JC):
                        nc.tensor.matmul(ps_[:], vin[:, k, tkc * 128:(tkc + 1) * 128],
                                         wva[:, k, :], start=(k == 0),
                                         stop=(k == JC - 1))
                    nc.scalar.copy(vj[:, tkc, :], ps_[:])

                for s in range(BL):
                    attOj = sp.tile([128, JC, L], F32R, tag="attOj", bufs=2,
                                    name=f"aoj{s}")

                    def kslice(h, kc, s=s):
                        hc, hp = JPIECE[h]
                        return [(kj[hp:hp + 64, hc,
                                    s * L + kc * 128:s * L + (kc + 1) * 128],
                                 qj[hp:hp + 64, hc, s * L:(s + 1) * L])]

                    def vcol(kc, c, po, sz, s=s):
                        return vj[:, s * 2 + kc, c * 128 + po:c * 128 + po + sz]

                    _softmax_attn(nc, sp, pp, pq, ones, attOj,
                                  lambda h: [(JPIECE[h][0], JPIECE[h][1], 64)],
                                  kslice, vcol, "a")
                    with nc.allow_low_precision(reason="attn q-sum"):
                        nc.vector.reduce_sum(attm_a[:, :, s:s + 1], attOj[:],
                                             axis=AX.X)
                for m in range(JC):
                    ps_ = pp.tile([128, BL], F32, tag="mm")
                    for k in range(JC):
                        nc.tensor.matmul(ps_[:], woa[:, k, m * 128:(m + 1) * 128],
                                         attm_a[:, k, :], start=(k == 0),
                                         stop=(k == JC - 1))
                    tmp_m = sp.tile([128, BL], F32, tag="tmpm", bufs=2,
                                    name=f"tmpm{m}")
                    nc.scalar.activation(tmp_m[:], ps_[:], AF.Identity,
                                         bias=boa[:, m:m + 1], scale=1.0 / L)
                    nc.vector.tensor_tensor(z1[:, m, :], tmp_m[:], x1a[:, m, :],
                                            ALU.add)

            # ---------------- stage E2: i2t cross attention (full) + patches
            with tc.tile_pool(name="w_b", bufs=1) as wp, \
                 tc.tile_pool(name="s_b", bufs=1) as sp:
                _ppctx = tc.tile_pool(name="pp_b", bufs=2, space="PSUM")
                pp = _ppctx.__enter__()
                jp = jnt["b"]
                wqb = wp.tile([128, JC, J], F32R)
                nc.sync.dma_start(wqb[:], jp["wq"].ap().rearrange("k p m -> p k m"))
                wkb = wp.tile([128, JC, J], F32R)
                nc.sync.dma_start(wkb[:], jp["wk"].ap().rearrange("k p m -> p k m"))
                wvb = wp.tile([128, JC, J], F32R)
                nc.sync.dma_start(wvb[:], jp["wv"].ap().rearrange("k p m -> p k m"))
                wob = wp.tile([128, JC, J], F32R)
                nc.sync.dma_start(wob[:], jp["wo"].ap().rearrange("k p m -> p k m"))
                bqb = wp.tile([128, JC], F32); nc.sync.dma_start(bqb[:], jp["bq"].ap())
                bob = wp.tile([128, JC], F32); nc.sync.dma_start(bob[:], jp["bo"].ap())
                qin2 = sp.tile([128, JC, T], F32R, tag="qin2")
                nc.sync.dma_start(qin2[:], dr["QI"].ap().rearrange("k p t -> p k t"))
                kin2 = sp.tile([128, JC, T], F32R, tag="kin2")
                nc.sync.dma_start(kin2[:], dr["KT"].ap().rearrange("k p t -> p k t"))
                vin2 = sp.tile([128, JC, T], F32R, tag="vin2")
                nc.sync.dma_start(vin2[:], dr["VT"].ap().rearrange("k p t -> p k t"))

                qj2 = sp.tile([128, JC, T], F32R, tag="qj2")
                kj2 = sp.tile([128, JC, T], F32R, tag="kj2")
                for m in range(JC):
                    for t4 in range(4):
                        tsl = slice(t4 * 512, (t4 + 1) * 512)
                        ps_ = pp.tile([128, 512], F32, tag="mm")
                        for k in range(JC):
                            nc.tensor.matmul(ps_[:], wqb[:, k, m * 128:(m + 1) * 128],
                                             qin2[:, k, tsl], start=(k == 0),
                                             stop=(k == JC - 1))
                        nc.scalar.activation(qj2[:, m, tsl], ps_[:], AF.Identity,
                                             bias=bqb[:, m:m + 1])
                        ps_ = pp.tile([128, 512], F32, tag="mm")
                        for k in range(JC):
                            nc.tensor.matmul(ps_[:], wkb[:, k, m * 128:(m + 1) * 128],
                                             kin2[:, k, tsl], start=(k == 0),
                                             stop=(k == JC - 1))
                        nc.scalar.copy(kj2[:, m, tsl], ps_[:])
                vj2 = sp.tile([128, 2 * BL, J], F32R, tag="vj2")
                for tkc in range(2 * BL):
                    ps_ = pp.tile([128, J], F32, tag="mm")
                    for k in range(JC):
                        nc.tensor.matmul(ps_[:], vin2[:, k, tkc * 128:(tkc + 1) * 128],
                                         wvb[:, k, :], start=(k == 0),
                                         stop=(k == JC - 1))
                    nc.scalar.copy(vj2[:, tkc, :], ps_[:])

                atb = sp.tile([128, JC, T], F32R, tag="atb")
                for s in range(BL):
                    attOj = sp.tile([128, JC, L], F32R, tag="attOj2", bufs=2,
                                    name=f"aoj2_{s}")

                    def kslice(h, kc, s=s):
                        hc, hp = JPIECE[h]
                        return [(kj2[hp:hp + 64, hc,
                                     s * L + kc * 128:s * L + (kc + 1) * 128],
                                 qj2[hp:hp + 64, hc, s * L:(s + 1) * L])]

                    def vcol(kc, c, po, sz, s=s):
                        return vj2[:, s * 2 + kc, c * 128 + po:c * 128 + po + sz]

                    _softmax_attn(nc, sp, pp, pq, ones, attOj,
                                  lambda h: [(JPIECE[h][0], JPIECE[h][1], 64)],
                                  kslice, vcol, "b")
                    for m in range(JC):
                        ps_ = pp.tile([128, L], F32, tag="mm")
                        for k in range(JC):
                            nc.tensor.matmul(ps_[:], wob[:, k, m * 128:(m + 1) * 128],
                                             attOj[:, k, :], start=(k == 0),
                                             stop=(k == JC - 1))
                        with nc.allow_low_precision(reason="i2t q-sum"):
                            nc.scalar.activation(atb[:, m, s * L:(s + 1) * L], ps_[:],
                                                 AF.Identity, bias=bob[:, m:m + 1],
                                                 accum_out=attm_b[:, m, s:s + 1])
                for m in range(JC):
                    nc.vector.scalar_tensor_tensor(z2[:, m, :], attm_b[:, m, :],
                                                   1.0 / L, tcls[:, m, :],
                                                   ALU.mult, ALU.add)

                # final LNs over 256 features (materialized; outputs are tiny)
                with tc.tile_pool(name="s_f", bufs=1) as sf:
                    ln1gt = sf.tile([128, JC], F32); nc.sync.dma_start(ln1gt[:], ln1g.ap())
                    ln1bt = sf.tile([128, JC], F32); nc.sync.dma_start(ln1bt[:], ln1b.ap())
                    ln2gt = sf.tile([128, JC], F32); nc.sync.dma_start(ln2gt[:], ln2g.ap())
                    ln2bt = sf.tile([128, JC], F32); nc.sync.dma_start(ln2bt[:], ln2b.ap())
                    for zin, gt, bt, xout, tg in ((z1, ln1gt, ln1bt, x1t, "l1"),
                                                  (z2, ln2gt, ln2bt, x2t, "l2")):
                        rstd, b2 = _ln_stats(nc, sf, pq, zin, JC, BL, ones,
                                             cmagic, tg)
                        ps_m = pq.tile([128, BL], F32, tag="bc")
                        nc.tensor.matmul(ps_m[:], ones[0:1, :], rstd[:],
                                         start=True, stop=True)
                        ps_b = pq.tile([128, BL], F32, tag="bc")
                        nc.tensor.matmul(ps_b[:], ones[0:1, :], b2[:],
                                         start=True, stop=True)
                        for m in range(JC):
                            tmp = sf.tile([128, BL], F32, tag=tg + "_t",
                                          name=f"{tg}tmp{m}")
                            nc.vector.tensor_tensor(tmp[:], zin[:, m, :], ps_m[:],
                                                    ALU.mult)
                            nc.vector.tensor_tensor(tmp[:], tmp[:], ps_b[:],
                                                    ALU.add)
                            nc.scalar.activation(xout[:, m, :], tmp[:], AF.Identity,
                                                 bias=bt[:, m:m + 1],
                                                 scale=gt[:, m:m + 1])

                _ppctx.__exit__(None, None, None)
                # ---------------- output assembly (transpose to token-major)
                with tc.tile_pool(name="s_o", bufs=3) as so, \
                     tc.tile_pool(name="pp_o", bufs=2, space="PSUM") as po:
                    ppatch = sp.tile([128, JC, T], F32R, tag="ppt")
                    nc.sync.dma_start(ppatch[:],
                                      dr["PP"].ap().rearrange("k p t -> p k t"))
                    fusec = sp.tile([128, JC, T], F32R, tag="fusec")
                    nc.vector.tensor_tensor(fusec[:], ppatch[:], atb[:], ALU.add)
                    for tkc in range(2 * BL):
                        s = tkc // 2
                        row0 = 1 + (tkc % 2) * 128
                        stg = so.tile([128, J], F32, tag="stg")
                        for c in range(JC):
                            pt = po.tile([128, 128], F32R, tag="tp")
                            nc.tensor.transpose(
                                pt[:], fusec[:, c, tkc * 128:(tkc + 1) * 128],
                                identr[:])
                            nc.scalar.copy(stg[:, c * 128:(c + 1) * 128], pt[:])
                        nc.sync.dma_start(out_d[s, row0:row0 + 128, :], stg[:])
                    x1s = so.tile([BL, J], F32, tag="xs")
                    x2s = so.tile([BL, J], F32, tag="xs")
                    for c in range(JC):
                        pt = po.tile([BL, 128], F32R, tag="tp")
                        nc.tensor.transpose(pt[:], x1t[:, c, :], identr[:])
                        nc.scalar.copy(x1s[:, c * 128:(c + 1) * 128], pt[:])
                        pt = po.tile([BL, 128], F32R, tag="tp")
                        nc.tensor.transpose(pt[:], x2t[:, c, :], identr[:])
                        nc.scalar.copy(x2s[:, c * 128:(c + 1) * 128], pt[:])
                    nc.sync.dma_start(out_d[:, 0, :], x1s[:])
                    nc.sync.dma_start(out_d[:, 1 + NPATCH, :], x2s[:])

    nc.compile()
    return nc


def _pack_params(params):
    """Host-side packing + algebraic folds of the parameter pytree."""
    def A(x):
        return np.asarray(x, dtype=np.float32)

    out = {}
    s192 = 1.0 / math.sqrt(DH)
    alphas = []
    ln_g, ln_b = {}, {}
    for pre, key in (("t", "txt_enh"), ("i", "ip_enh"), ("g", "ig_enh")):
        e = params[key]
        ln_g[pre] = A(e["ln"]["g"])
        ln_b[pre] = A(e["ln"]["b"])
        alphas.append(float(A(e["alpha"]).reshape(-1)[0]))

    for pre, key in (("t", "txt_enh"), ("i", "ip_enh")):
        e = params[key]
        m = e["mha"]
        out[f"{pre}_wq"] = _pack_wT(A(m["q"]["W"]) * s192)
        out[f"{pre}_bq"] = _pack_b(A(m["q"]["b"]) * s192)
        out[f"{pre}_wk"] = _pack_wT(A(m["k"]["W"]))
        out[f"{pre}_wv"] = _pack_wT(A(m["v"]["W"]))
        out[f"{pre}_wo"] = _pack_wT(A(m["o"]["W"]))
        out[f"{pre}_bo"] = _pack_b(A(m["o"]["W"]) @ A(m["v"]["b"]) + A(m["o"]["b"]))
        pos = A(e["pos"])[0, :L]
        out[f"{pre}_pos"] = np.ascontiguousarray(pos.T.reshape(DC, 128, L))
    g = params["ig_enh"]
    gm = g["mha"]
    out["g_w"] = _pack_wT(A(gm["o"]["W"]) @ A(gm["v"]["W"]))
    out["g_bo"] = _pack_b(A(gm["o"]["W"]) @ A(gm["v"]["b"]) + A(gm["o"]["b"]))
    gpos = A(g["pos"])[0, 0]
    out["g_pos"] = np.ascontiguousarray(
        np.repeat(gpos.reshape(DC, 128, 1), BL, axis=2))

    # cross projections: fold LN gain into W (W' = W*diag(g)); rank-1 u = W'@1;
    # the constant part (W@b_ln + b) folds into downstream biases.
    def fold_cross(wkey, src_ln):
        W = A(params[wkey]["W"])
        bc = A(params[wkey]["b"])
        Wp = W * ln_g[src_ln][None, :]
        u = Wp.sum(axis=1)
        vconst = W @ ln_b[src_ln] + bc
        return Wp, u, vconst

    vconsts = {}
    for name, wkey, src in (("c_qt", "query_txt", "t"), ("c_kt", "key_txt", "t"),
                            ("c_vt", "value_txt", "t"), ("c_ki", "key_img", "i"),
                            ("c_vi", "value_img", "i"), ("c_qi", "query_img", "i"),
                            ("c_pp", "img_patch_proj", "i"),
                            ("c_gp", "img_global_proj", "g"),
                            ("c_tp", "txt_proj", "t")):
        Wp, u, vconst = fold_cross(wkey, src)
        out[name] = _pack_wT(Wp)
        out[name + "_u"] = np.ascontiguousarray(u[None, :])
        vconsts[name] = vconst
    out["c_pp_b"] = _pack_b(vconsts["c_pp"])
    out["c_gp_b"] = _pack_b(vconsts["c_gp"])
    out["c_tp_b"] = _pack_b(vconsts["c_tp"])

    s64 = 1.0 / math.sqrt(J // H)
    for pre, key, qv, vv in (("a", "attn_t2i", "c_qt", "c_vi"),
                             ("b", "attn_i2t", "c_qi", "c_vt")):
        m = params[key]
        out[f"{pre}_wq"] = _pack_wT(A(m["q"]["W"]) * s64)
        out[f"{pre}_bq"] = _pack_b(
            (A(m["q"]["W"]) @ vconsts[qv] + A(m["q"]["b"])) * s64)
        out[f"{pre}_wk"] = _pack_wT(A(m["k"]["W"]))
        out[f"{pre}_wv"] = _pack_wT(A(m["v"]["W"]))
        out[f"{pre}_wo"] = _pack_wT(A(m["o"]["W"]))
        out[f"{pre}_bo"] = _pack_b(
            A(m["o"]["W"]) @ (A(m["v"]["W"]) @ vconsts[vv] + A(m["v"]["b"]))
            + A(m["o"]["b"]))
    out["ln1g"] = _pack_b(A(params["ln_img"]["g"]))
    out["ln1b"] = _pack_b(A(params["ln_img"]["b"]))
    out["ln2g"] = _pack_b(A(params["ln_txt"]["g"]))
    out["ln2b"] = _pack_b(A(params["ln_txt"]["b"]))
    out["ones"] = np.ones((128, 128), np.float32)
    out["identr"] = np.eye(128, dtype=np.float32)
    return out, tuple(alphas)


def kernel(img_global, img_patch, txt_feats, params):
    img_global = np.asarray(img_global, np.float32)
    img_patch = np.asarray(img_patch, np.float32)
    txt_feats = np.asarray(txt_feats, np.float32)
    packed, alphas = _pack_params(params)

    if alphas not in _prog_cache:
        _prog_cache[alphas] = _build_program(*alphas)
    nc = _prog_cache[alphas]

    in_maps = []
    for c in range(N_CORES):
        sl = slice(c * BL, (c + 1) * BL)
        m = dict(packed)
        m["xt"] = _pack_xT(txt_feats[sl])
        m["xi"] = _pack_xT(img_patch[sl])
        m["xg"] = np.ascontiguousarray(img_global[sl].T.reshape(DC, 128, BL))
        in_maps.append(m)

    import os
    kwargs = {}
    if os.environ.get("BASS_KERNEL_TRACE"):
        import types
        import antenv
        if "antenv.axon_hooks" not in sys.modules:
            _m = types.ModuleType("antenv.axon_hooks")
            _h = [None]
            _m.set_axon_ntff_profile_hook = lambda h: _h.__setitem__(0, h)
            _m.get_axon_ntff_profile_hook = lambda: _h[0]
            sys.modules["antenv.axon_hooks"] = _m
            antenv.axon_hooks = _m
            from trn_agent_boot.trn_boot import _ntff_profile_via_ctypes
            _m.set_axon_ntff_profile_hook(
                _ntff_profile_via_ctypes("/opt/axon/libaxon_pjrt.so"))
        kwargs = dict(trace=True, tmpdir=os.environ["BASS_KERNEL_TRACE"])

    res = run_bass_kernel_spmd(nc, in_maps, list(range(N_CORES)), **kwargs)
    kernel.last_results = res
    out = np.concatenate([res.results[c]["out"] for c in range(N_CORES)], axis=0)
    return np.ascontiguousarray(out, dtype=np.float32)


kernel.last_results = None
